# revision 21
# baseline (speedup 1.0000x reference)
"""CosineCrossAttention Trainium2 kernel (v4).

Math (per (b,t)):
    q = query @ Wq                      (N, D), heads head-major: d = h*48+dh
    k = kv @ Wk   (1, D);  v = kv @ Wv  (1, D)
    attn[n,h] = (q_h . k_h) / (|q_h||k_h|)
    out[n, dh*8+h] = attn[n,h] * v[h,dh];  out = out @ Wp + bp

Restructured:
    Wqk[d,t,h]  = (Wq @ (k_t masked per head))          attn_raw = query @ Wqk
    ss[n,h]     = sum_{d in head h} q[n,d]^2   (mask48 matmul on q^2)
    attn        = attn_raw * rsqrt(ss);  1/|k_h| folded into Wp_eff
    Wp_eff[h,:] = sum_d v_perm[d]*(d%8==h)*Wp[d,:]
    out         = attn @ Wp_eff         (bp added on host; zeros anyway)

v5 performance structure (per 512-col group; 32 groups/core):
  - qproj fp8 DoubleRow (6 MMs) + ss fp8 DR (2 MMs, pq-pool bank) as in v3.
  - attn stationary (wqk) and the ss masks are column-REPLICATED to array
    strips {0-7, 32-39, 64-71}, so attn_raw and ss land in 3 partition
    strips of psum directly; the rcp/sqrt/mult norm chain runs ONCE on
    partitions [0:72] (DVE/ACT cost is free-dim-driven, so this costs the
    same as [0:8]) producing att pre-replicated for the row-tiled out MMs.
    ss mask gap columns hold a copy of head 0's mask to avoid 1/0=inf.
  - out-projection row-tiled: 3 K=8 MMs run CONCURRENTLY in 32-row strips
    of the PE array (auto tile_position from lhsT base partition),
    writing a single merged [128,3,512] psum tile (3 banks).
  - merged out-evac: one DVE copy [:, :, :448] + one ACT copy [:, :, 448:]
    (engine-balanced; DVE/ACT psum op cost = (120|172 + FD)/f).
  - PSUM budget exactly 8 banks: pq ring 3 (qproj chunks + ss share it),
    par 2, po 3 (single merged buf).
  - head: PE warmup MMs (HAM un-throttle) while c8+qt8 stream in; cb16
    const blob split into 3 prioritized DMAs; qt tiles prefetched one
    (t,half) tile ahead so qproj never waits on its own DMA.
  - out + replicate DMAs issue from the (otherwise idle) gpsimd queue.

Sharding: data-parallel over B across the 8 cores (one batch element each).
"""

import sys

sys.path.insert(0, "/opt/trn_rl_repo")

from contextlib import ExitStack

import ml_dtypes
import numpy as np

import concourse.tile as tile
from concourse import bacc, mybir

F32 = mybir.dt.float32
BF16 = mybir.dt.bfloat16
FP8 = mybir.dt.float8e4
DR = mybir.MatmulPerfMode.DoubleRow

B, T, N, D, H, Dh = 8, 8, 2048, 384, 8, 48
P = 128
CH = D // P  # 3 chunks of the D dims
NG = 512  # n-group (one PSUM bank of f32)
ES = 448  # evac split point: DVE takes [0:ES], ACT takes [ES:NG]

# bf16 const blob offsets (in bf16 elements per partition), priority-ordered:
# part A (preamble k-path), part B (Wq^T for wqk), part C (v-path + Wp)
_WK = 0
_KVT = _WK + CH * D
_M48 = _KVT + CH * T  # t_dim-dependent; recomputed in _offsets()
_MV = 0
_WQT = 0
_WVP = 0
_WP = 0


def _offsets(t_dim):
    wk = 0
    kvt = wk + CH * D
    a_end = kvt + CH * t_dim
    m48 = a_end
    mv = m48 + CH * H
    wqt = mv + CH * H
    b_end = wqt + CH * D
    wvp = b_end
    wp = wvp + CH * D
    total = wp + CH * D
    return dict(WK=wk, KVT=kvt, M48=m48, MV=mv, A=a_end, WQT=wqt, B=b_end,
                WVP=wvp, WP=wp, TOTAL=total)


def build_nc(t_dim=T, n_dim=N):
    nc = bacc.Bacc("TRN2", target_bir_lowering=False, debug=False)
    off = _offsets(t_dim)

    nh = min(2 * NG, n_dim)  # columns per qt tile (half-t)
    gph = nh // NG  # groups per half
    nhalves = n_dim // nh
    groups = [
        (t, hf, gl) for t in range(t_dim) for hf in range(nhalves) for gl in range(gph)
    ]
    tiles_order = [(t, hf) for t in range(t_dim) for hf in range(nhalves)]
    G = len(groups)

    qT = nc.dram_tensor("qT", [t_dim, D, n_dim], BF16, kind="ExternalInput").ap()
    qT8 = nc.dram_tensor("qT8", [t_dim, D, n_dim], FP8, kind="ExternalInput").ap()
    cb16_d = nc.dram_tensor("cb16", [P, off["TOTAL"]], BF16, kind="ExternalInput").ap()
    c8_d = nc.dram_tensor("c8", [P, CH * D + CH * P], FP8, kind="ExternalInput").ap()
    outT = nc.dram_tensor("outT", [t_dim, D, n_dim], BF16, kind="ExternalOutput").ap()

    with tile.TileContext(nc) as tc, ExitStack() as ctx:
        consts = ctx.enter_context(tc.tile_pool(name="consts", bufs=1))
        qpool = ctx.enter_context(tc.tile_pool(name="qpool", bufs=6))
        q8pool = ctx.enter_context(tc.tile_pool(name="q8pool", bufs=6))
        work = ctx.enter_context(tc.tile_pool(name="work", bufs=2))
        qsqp = ctx.enter_context(tc.tile_pool(name="qsqp", bufs=3))
        nrmp = ctx.enter_context(tc.tile_pool(name="nrmp", bufs=2))
        attp = ctx.enter_context(tc.tile_pool(name="attp", bufs=4))
        osbp = ctx.enter_context(tc.tile_pool(name="osbp", bufs=6))
        pqp = ctx.enter_context(tc.tile_pool(name="pqp", bufs=3, space="PSUM"))
        ssp = ctx.enter_context(tc.tile_pool(name="ssp", bufs=1, space="PSUM"))
        parp = ctx.enter_context(tc.tile_pool(name="parp", bufs=1, space="PSUM"))
        pop = ctx.enter_context(tc.tile_pool(name="pop", bufs=1, space="PSUM"))
        dram = ctx.enter_context(tc.tile_pool(name="dram", bufs=1, space="DRAM"))

        # ---------- const + first-tile DMAs, priority order ----------
        cb16_t = consts.tile([P, off["TOTAL"]], BF16, tag="cb16")
        c8_t = consts.tile([P, CH * D + CH * P], FP8, tag="c8")
        wq8v = c8_t[:, 0 : CH * D].rearrange("p (c d) -> p c d", c=CH)
        # m48 padded to 128 stationary columns (cols 8.. are zero): DoubleRow
        # ldweights is only valid for full-width (0,0) tiles.
        m48v = c8_t[:, CH * D :].rearrange("p (c m) -> p c m", c=CH)

        qt_tiles = {}
        qt_parts = {}

        def emit_qdma(h, part=None):
            # part=None: whole tile; part=0: qt8 + first half of qt;
            # part=1: second half of qt (issued a round later to smooth
            # per-DMA-queue bursts that delay interleaved output DMAs)
            t, hf = h
            sl = slice(hf * nh, (hf + 1) * nh)
            if part in (None, 0):
                qt8 = q8pool.tile([P, CH, nh], FP8, tag="qt8")
                nc.sync.dma_start(
                    qt8, qT8[t].rearrange("(c p) n -> p c n", p=P)[:, :, sl]
                )
                qt = qpool.tile([P, CH, nh], BF16, tag="qt")
                qt_tiles[h] = (qt, qt8)
            qt = qt_tiles[h][0]
            hw = nh // 2
            src3 = qT[t].rearrange("(c p) n -> p c n", p=P)[:, :, sl]
            if part is None:
                nc.sync.dma_start(qt, src3)
            elif part == 0:
                nc.sync.dma_start(qt[:, :, 0:hw], src3[:, :, 0:hw])
            else:
                nc.sync.dma_start(qt[:, :, hw:], src3[:, :, hw:])
            qt_parts[h] = part

        emit_qdma(tiles_order[0])
        nc.sync.dma_start(c8_t, c8_d)
        nc.sync.dma_start(cb16_t[:, : off["A"]], cb16_d[:, : off["A"]])
        nc.sync.dma_start(cb16_t[:, off["A"] : off["B"]], cb16_d[:, off["A"] : off["B"]])

        def W(o, c, w):
            base = o + c * w
            return cb16_t[:, base : base + w]

        # ---------- PE warmup (HAM un-throttle while DMAs land) ----------
        wz = consts.tile([P, P], BF16, tag="wz")
        nc.vector.memset(wz, 0.0)
        # dummy sqrt: hoists the sqrt ACT-table load off the first round's
        # critical path (tables load on first use per function)
        dumq = consts.tile([1, 8], F32, tag="dumq")
        nc.vector.memset(dumq, 1.0)
        nc.scalar.sqrt(dumq, dumq)
        warm = pop.tile([H, P], F32, tag="po")
        for _ in range(24):
            nc.tensor.matmul(
                warm, wz[:, 0:H], wz, start=True, stop=True, skip_group_check=True
            )

        for k in (1, 2):
            if k < len(tiles_order):
                emit_qdma(tiles_order[k])
        nc.sync.dma_start(cb16_t[:, off["B"] :], cb16_d[:, off["B"] :])
        for k in (3, 4, 5):
            if k < len(tiles_order):
                emit_qdma(tiles_order[k])

        # ---------- per-group emission helpers ----------
        par_tiles = {}
        qsq_tiles = {}
        att_tiles = {}

        def emit_qproj_open(g):
            par = parp.tile([P, NG], F32, tag="par")
            qsq = qsqp.tile([P, CH, NG], FP8, tag="qsq")
            par_tiles[g] = par
            qsq_tiles[g] = qsq

        def emit_qproj_chunk(g, co):
            t, hf, gl = groups[g]
            qt, qt8 = qt_tiles[(t, hf)]
            qsl = slice(gl * NG, (gl + 1) * NG)
            qsq = qsq_tiles[g]
            pqc = pqp.tile([P, NG], F32, tag="pq")
            nc.tensor.matmul(
                pqc, wq8v[:, 0:2, co * P : (co + 1) * P], qt8[:, 0:2, qsl],
                start=True, stop=False, perf_mode=DR, skip_group_check=True,
            )
            nc.tensor.matmul(
                pqc, wq8v[:, 2, co * P : (co + 1) * P], qt8[:, 2, qsl],
                start=False, stop=True, skip_group_check=True,
            )
            nc.scalar.square(qsq[:, co, :], pqc)

        def emit_qproj(g):
            emit_qproj_open(g)
            for co in range(CH):
                emit_qproj_chunk(g, co)

        RW = 64 + H  # replicated stationary width: strips at 0, 32, 64

        def emit_attn(g):
            t, hf, gl = groups[g]
            qt, _ = qt_tiles[(t, hf)]
            qsl = slice(gl * NG, (gl + 1) * NG)
            par = par_tiles[g]
            for c in range(CH):
                nc.tensor.matmul(
                    par[0:RW, :], wqk[:, c, t, :], qt[:, c, qsl],
                    start=(c == 0), stop=(c == CH - 1),
                    tile_position=(0, 0),
                )

        rs_tiles = {}

        def emit_stageA1(g):
            qsq = qsq_tiles.pop(g)
            sspq = ssp.tile([P, NG], F32, tag="ss")
            nc.tensor.matmul(
                sspq, m48v[:, 0:2, :], qsq[:, 0:2, :],
                start=True, stop=False, perf_mode=DR, skip_group_check=True,
            )
            nc.tensor.matmul(
                sspq, m48v[:, 2, :], qsq[:, 2, :],
                start=False, stop=True, skip_group_check=True,
            )
            # rcp first: frees the ss psum bank after a single DVE op
            rs = nrmp.tile([RW, NG], F32, tag="rs")
            nc.vector.reciprocal_approx_fast(rs, sspq[0:RW, :])
            rs_tiles[g] = rs

        def emit_stageA2(g):
            # att = attn_raw * sqrt(1/ss); 1/|k_h| is folded into Wp_eff.
            # Everything is strip-replicated on partitions [0:72], so att
            # comes out pre-replicated for the row-tiled out MMs.
            par = par_tiles.pop(g)
            rs = rs_tiles.pop(g)
            nc.scalar.sqrt(rs, rs)
            att = attp.tile([P, NG], BF16, tag="att")
            nc.vector.tensor_tensor(att[0:RW, :], par[0:RW, :], rs, op=mybir.AluOpType.mult)
            att_tiles[g] = att

        def emit_stageA(g):
            emit_stageA1(g)
            emit_stageA2(g)

        osb_cur = [None]

        def emit_stageB(g):
            t, hf, gl = groups[g]
            att = att_tiles.pop(g)
            po = pop.tile([P, CH, NG], F32, tag="po")
            for r in range(CH):
                nc.tensor.matmul(
                    po[:, r, :],
                    wpe_srep[32 * r : 32 * r + H, t, r * P : (r + 1) * P],
                    att[32 * r : 32 * r + H, :],
                    start=True, stop=True, skip_group_check=True,
                )
            # pair-batched osb: both groups of a (t,hf) tile share one tile,
            # flushed with a single 2KB-row DMA (DMA descriptor efficiency)
            if gl == 0 or osb_cur[0] is None:
                osbt = osbp.tile([P, CH, gph * NG], BF16, tag="osb")
                osb_cur[0] = osbt
            osb = osb_cur[0]
            gsl = slice(gl * NG, (gl + 1) * NG)
            nc.scalar.copy(osb[:, :, gl * NG + ES : (gl + 1) * NG], po[:, :, ES:])
            nc.vector.tensor_copy(osb[:, :, gl * NG : gl * NG + ES], po[:, :, 0:ES])
            if gl == gph - 1:
                sl = slice(hf * nh, (hf + 1) * nh)
                dst = outT[t].rearrange("(c p) n -> p c n", p=P)[:, :, sl]
                nc.gpsimd.dma_start(dst, osb)
                osb_cur[0] = None

        # ---------- preamble ----------
        PRE = min(2, G)
        emit_qproj(0)

        # k projection computed directly in transposed (D-part, t) form:
        # stationary = weight chunk, moving = kvT (t rows) - no transposes.
        kT = consts.tile([P, CH, t_dim], BF16, tag="kT")
        for co in range(CH):
            pk = pqp.tile([P, t_dim], F32, tag="pq")
            for c in range(CH):
                nc.tensor.matmul(
                    pk, W(off["WK"], c, D)[:, co * P : (co + 1) * P],
                    W(off["KVT"], c, t_dim),
                    start=(c == 0), stop=(c == CH - 1),
                )
            nc.vector.tensor_copy(kT[:, co, :], pk)

        # per-head k norms: rnkT[h, t] = 1/|k_h|(t)
        ksqT = work.tile([P, CH, t_dim], BF16, tag="ksqT")
        nc.scalar.square(ksqT, kT)
        psk2 = pqp.tile([H, t_dim], F32, tag="pq")
        for c in range(CH):
            nc.tensor.matmul(
                psk2, W(off["M48"], c, H), ksqT[:, c, :],
                start=(c == 0), stop=(c == CH - 1),
            )
        rnkT = consts.tile([H, t_dim], F32, tag="rnkT")
        nc.scalar.sqrt(rnkT, psk2)
        nc.vector.reciprocal(rnkT, rnkT)

        if PRE > 1:
            emit_qproj(1)

        # Kmat[d, t, h] = kT[d, t] * m48[d, h]
        m48b = cb16_t[:, off["M48"] : off["M48"] + CH * H].rearrange(
            "p (c h) -> p c h", c=CH
        )
        mvb = cb16_t[:, off["MV"] : off["MV"] + CH * H].rearrange(
            "p (c h) -> p c h", c=CH
        )
        kmat = consts.tile([P, CH, t_dim, H], BF16, tag="kmat")
        for h in range(H):
            nc.vector.tensor_tensor(
                kmat[:, :, :, h],
                kT,
                m48b[:, :, h : h + 1].to_broadcast((P, CH, t_dim)),
                op=mybir.AluOpType.mult,
            )

        # Wqk[d_in, t, h] = sum_dmid Wq[d_in, dmid] Kmat[dmid, t, h]
        # Stationary columns replicated to strips {0-7, 32-39, 64-71} so the
        # attn matmuls write all three partition strips at once; gaps zero.
        wqk = consts.tile([P, CH, t_dim, RW], BF16, tag="wqk")
        nc.gpsimd.memset(wqk, 0.0)
        for ci in range(CH):
            pw = pqp.tile([P, t_dim * H], F32, tag="pq")
            for cm in range(CH):
                nc.tensor.matmul(
                    pw,
                    W(off["WQT"], cm, D)[:, ci * P : (ci + 1) * P],
                    kmat[:, cm, :, :],
                    start=(cm == 0), stop=(cm == CH - 1),
                )
            pwv = pw.rearrange("p (t h) -> p t h", h=H)
            for s in range(3):
                nc.vector.tensor_copy(wqk[:, ci, :, 32 * s : 32 * s + H], pwv)

        for g in range(PRE):
            emit_attn(g)

        # v projection (transposed) + Vsel
        vT = consts.tile([P, CH, t_dim], BF16, tag="vT")
        for co in range(CH):
            pv = pqp.tile([P, t_dim], F32, tag="pq")
            for c in range(CH):
                nc.tensor.matmul(
                    pv, W(off["WVP"], c, D)[:, co * P : (co + 1) * P],
                    W(off["KVT"], c, t_dim),
                    start=(c == 0), stop=(c == CH - 1),
                )
            nc.vector.tensor_copy(vT[:, co, :], pv)
        vsel = consts.tile([P, CH, t_dim, H], BF16, tag="vsel")
        for h in range(H):
            nc.vector.tensor_tensor(
                vsel[:, :, :, h],
                vT,
                mvb[:, :, h : h + 1].to_broadcast((P, CH, t_dim)),
                op=mybir.AluOpType.mult,
            )

        # Wp_eff[(t,h), d_out] = sum_d Vsel[d, t, h] * Wp[d, d_out]
        pe_all = pop.tile([t_dim * H, D], F32, tag="po")
        for c in range(CH):
            nc.tensor.matmul(
                pe_all, vsel[:, c].rearrange("p t h -> p (t h)"), W(off["WP"], c, D),
                start=(c == 0), stop=(c == CH - 1),
            )
        wpe_stage = work.tile([t_dim * H, D], BF16, tag="wpestage")
        nc.scalar.copy(wpe_stage, pe_all)
        wpe_dram = dram.tile([t_dim * H, D], BF16)
        nc.gpsimd.dma_start(wpe_dram, wpe_stage)
        wpe = consts.tile([H, t_dim, D], BF16, tag="wpe")
        nc.gpsimd.dma_start(wpe, wpe_dram.rearrange("(t h) d -> h t d", h=H))
        # fold 1/|k_h| into the output projection; replicate to row strips
        wpe_srep = consts.tile([P, t_dim, D], BF16, tag="wpes")
        nc.vector.tensor_tensor(
            wpe_srep[0:H], wpe, rnkT[:, :, None].to_broadcast((H, t_dim, D)),
            op=mybir.AluOpType.mult,
        )
        nc.gpsimd.dma_start(wpe_srep[32 : 32 + H], wpe_srep[0:H])
        nc.gpsimd.dma_start(wpe_srep[64 : 64 + H], wpe_srep[0:H])

        if PRE > 1:
            emit_stageA(0)

        # ---------- software-pipelined main loop ----------
        LAGB = 2
        NTAIL = min(4, G)  # last groups drain per-chunk through the pq ring
        if (G - NTAIL) % gph:
            NTAIL += 1  # keep the body/tail boundary pair-aligned
        for g in range(PRE, G):
            t, hf, gl = groups[g]
            if gl == 0:
                k = g // gph + 4  # prefetch four (t,half) tiles ahead
                if k < len(tiles_order) and tiles_order[k] not in qt_tiles:
                    emit_qdma(tiles_order[k])
            emit_stageA1(g - 1)
            if 0 <= g - LAGB < G - NTAIL:
                emit_stageB(g - LAGB)
            emit_qproj_open(g)
            emit_qproj_chunk(g, 0)
            emit_qproj_chunk(g, 1)
            emit_stageA2(g - 1)
            emit_qproj_chunk(g, 2)
            emit_attn(g)
        def emit_stageB_tail(g):
            t, hf, gl = groups[g]
            sl = slice(hf * nh + gl * NG, hf * nh + (gl + 1) * NG)
            att = att_tiles.pop(g)
            osb = osbp.tile([P, CH, NG], BF16, tag="osb")
            dst = outT[t].rearrange("(c p) n -> p c n", p=P)[:, :, sl]
            for r in range(CH):
                poc = pqp.tile([P, NG], F32, tag="pq")
                nc.tensor.matmul(
                    poc,
                    wpe_srep[32 * r : 32 * r + H, t, r * P : (r + 1) * P],
                    att[32 * r : 32 * r + H, :],
                    start=True, stop=True, skip_group_check=True,
                )
                if r == 1:
                    nc.vector.tensor_copy(osb[:, r, :], poc)
                else:
                    nc.scalar.copy(osb[:, r, :], poc)
                eng = (nc.gpsimd, nc.sync, nc.gpsimd)[r]
                eng.dma_start(dst[:, r, :], osb[:, r, :])

        emit_stageA(G - 1)
        for g in range(G - NTAIL, G):
            emit_stageB_tail(g)

    nc.compile()
    return nc


_CACHE = {}


def _get_nc(t_dim=T, n_dim=N):
    key = (t_dim, n_dim)
    if key not in _CACHE:
        _CACHE[key] = build_nc(t_dim, n_dim)
    return _CACHE[key]


def _host_prep(query, kv, Wq, Wk, Wv, Wp, bp):
    bf = ml_dtypes.bfloat16
    f8 = ml_dtypes.float8_e4m3fn
    query = np.asarray(query, dtype=np.float32)
    kv = np.asarray(kv, dtype=np.float32)
    Wq = np.asarray(Wq, dtype=np.float32)
    Wk = np.asarray(Wk, dtype=np.float32)
    Wv = np.asarray(Wv, dtype=np.float32)
    Wp = np.asarray(Wp, dtype=np.float32)
    bp = np.asarray(bp, dtype=np.float32)

    b_dim, t_dim, n_dim, d = query.shape
    dh = d // H
    off = _offsets(t_dim)

    def img(mat, width):  # [D, width] -> [P, CH*width] chunk-major image
        return np.ascontiguousarray(
            mat.reshape(CH, P, width).transpose(1, 0, 2).reshape(P, CH * width)
        )

    perm = (np.arange(d) % H) * dh + np.arange(d) // H
    Wvp = Wv[:, perm]
    dd = np.arange(d)
    hh = np.arange(H)
    m48 = (dd[:, None] // dh == hh[None, :]).astype(np.float32)
    mv = (dd[:, None] % H == hh[None, :]).astype(np.float32)

    def reg(base, w):
        return slice(base, base + w)

    base = np.empty((P, off["TOTAL"]), dtype=bf)
    base[:, reg(off["WK"], CH * d)] = img(Wk, d).astype(bf)
    base[:, reg(off["M48"], CH * H)] = img(m48, H).astype(bf)
    base[:, reg(off["MV"], CH * H)] = img(mv, H).astype(bf)
    base[:, reg(off["WQT"], CH * d)] = img(np.ascontiguousarray(Wq.T), d).astype(bf)
    base[:, reg(off["WVP"], CH * d)] = img(Wvp, d).astype(bf)
    base[:, reg(off["WP"], CH * d)] = img(Wp, d).astype(bf)

    # ss masks strip-replicated at cols {0-7, 32-39, 64-71}; every gap
    # column carries head 0's mask so 1/ss never sees a zero.
    m48pad = np.tile(m48[:, 0:1], (1, P)).astype(np.float32)
    for s in range(3):
        m48pad[:, 32 * s : 32 * s + H] = m48
    c8 = np.empty((P, CH * d + CH * P), dtype=f8)
    c8[:, : CH * d] = img(Wq, d).astype(f8)
    c8[:, CH * d :] = img(m48pad, P).astype(f8)

    in_maps = []
    for b in range(b_dim):
        cb16 = base.copy()
        cb16[:, reg(off["KVT"], CH * t_dim)] = img(kv[b, :, 0, :].T, t_dim).astype(bf)
        qTb = np.ascontiguousarray(query[b].transpose(0, 2, 1))
        in_maps.append(
            {
                "qT": qTb.astype(bf),
                "qT8": qTb.astype(f8),
                "cb16": cb16,
                "c8": c8,
            }
        )
    return in_maps, (b_dim, t_dim, n_dim, d), bp


def _gather(results, shape, bp):
    b_dim, t_dim, n_dim, d = shape
    out = np.empty((b_dim, t_dim, n_dim, d), dtype=np.float32)
    for b in range(b_dim):
        out[b] = results[b]["outT"].astype(np.float32).transpose(0, 2, 1)
    if np.any(bp):
        out += bp
    return out


def kernel(query, kv, Wq, Wk, Wv, Wp, bp):
    from concourse.bass_utils import run_bass_kernel_spmd

    in_maps, shape, bp = _host_prep(query, kv, Wq, Wk, Wv, Wp, bp)
    nc = _get_nc(shape[1], shape[2])
    res = run_bass_kernel_spmd(nc, in_maps, core_ids=list(range(len(in_maps))))
    return _gather(res.results, shape, bp)


def _install_ntff_hook():
    """The agent image's antenv lacks axon_hooks; synthesize it so
    run_bass_kernel_spmd(trace=True) can capture NTFF profiles."""
    import types

    if "antenv.axon_hooks" in sys.modules:
        return
    sys.path.insert(0, "/root/.axon_site")
    from trn_agent_boot.trn_boot import _ntff_profile_via_ctypes

    hook = _ntff_profile_via_ctypes("/opt/axon/libaxon_pjrt.so")
    mod = types.ModuleType("antenv.axon_hooks")
    mod.get_axon_ntff_profile_hook = lambda: hook
    mod.set_axon_ntff_profile_hook = lambda h: None
    sys.modules["antenv.axon_hooks"] = mod


def kernel_traced(query, kv, Wq, Wk, Wv, Wp, bp):
    """Like kernel() but captures an NTFF profile; returns (out, results)."""
    from concourse.bass_utils import run_bass_kernel_spmd

    _install_ntff_hook()
    in_maps, shape, bp = _host_prep(query, kv, Wq, Wk, Wv, Wp, bp)
    nc = _get_nc(shape[1], shape[2])
    res = run_bass_kernel_spmd(
        nc, in_maps, core_ids=list(range(len(in_maps))), trace=True
    )
    return _gather(res.results, shape, bp), res


# revision 22
# speedup vs baseline: 1.1064x; 1.1064x over previous
"""CosineCrossAttention Trainium2 kernel (v4).

Math (per (b,t)):
    q = query @ Wq                      (N, D), heads head-major: d = h*48+dh
    k = kv @ Wk   (1, D);  v = kv @ Wv  (1, D)
    attn[n,h] = (q_h . k_h) / (|q_h||k_h|)
    out[n, dh*8+h] = attn[n,h] * v[h,dh];  out = out @ Wp + bp

Restructured:
    Wqk[d,t,h]  = (Wq @ (k_t masked per head))          attn_raw = query @ Wqk
    ss[n,h]     = sum_{d in head h} q[n,d]^2   (mask48 matmul on q^2)
    attn        = attn_raw * rsqrt(ss);  1/|k_h| folded into Wp_eff
    Wp_eff[h,:] = sum_d v_perm[d]*(d%8==h)*Wp[d,:]
    out         = attn @ Wp_eff         (bp added on host; zeros anyway)

v5 performance structure (per 512-col group; 32 groups/core):
  - qproj fp8 DoubleRow (6 MMs) + ss fp8 DR (2 MMs, pq-pool bank) as in v3.
  - attn stationary (wqk) and the ss masks are column-REPLICATED to array
    strips {0-7, 32-39, 64-71}, so attn_raw and ss land in 3 partition
    strips of psum directly; the rcp/sqrt/mult norm chain runs ONCE on
    partitions [0:72] (DVE/ACT cost is free-dim-driven, so this costs the
    same as [0:8]) producing att pre-replicated for the row-tiled out MMs.
    ss mask gap columns hold a copy of head 0's mask to avoid 1/0=inf.
  - out-projection row-tiled: 3 K=8 MMs run CONCURRENTLY in 32-row strips
    of the PE array (auto tile_position from lhsT base partition),
    writing a single merged [128,3,512] psum tile (3 banks).
  - merged out-evac: one DVE copy [:, :, :448] + one ACT copy [:, :, 448:]
    (engine-balanced; DVE/ACT psum op cost = (120|172 + FD)/f).
  - PSUM budget exactly 8 banks: pq ring 3 (qproj chunks + ss share it),
    par 2, po 3 (single merged buf).
  - head: PE warmup MMs (HAM un-throttle) while c8+qt8 stream in; cb16
    const blob split into 3 prioritized DMAs; qt tiles prefetched one
    (t,half) tile ahead so qproj never waits on its own DMA.
  - out + replicate DMAs issue from the (otherwise idle) gpsimd queue.

Sharding: data-parallel over B across the 8 cores (one batch element each).
"""

import sys

sys.path.insert(0, "/opt/trn_rl_repo")

from contextlib import ExitStack

import ml_dtypes
import numpy as np

import concourse.tile as tile
from concourse import bacc, mybir

F32 = mybir.dt.float32
BF16 = mybir.dt.bfloat16
FP8 = mybir.dt.float8e4
DR = mybir.MatmulPerfMode.DoubleRow

B, T, N, D, H, Dh = 8, 8, 2048, 384, 8, 48
P = 128
CH = D // P  # 3 chunks of the D dims
NG = 512  # n-group (one PSUM bank of f32)
ES = 448  # evac split point: DVE takes [0:ES], ACT takes [ES:NG]

# bf16 const blob offsets (in bf16 elements per partition), priority-ordered:
# part A (preamble k-path), part B (Wq^T for wqk), part C (v-path + Wp)
_WK = 0
_KVT = _WK + CH * D
_M48 = _KVT + CH * T  # t_dim-dependent; recomputed in _offsets()
_MV = 0
_WQT = 0
_WVP = 0
_WP = 0


def _offsets(t_dim):
    wk = 0
    kvt = wk + CH * D
    a_end = kvt + CH * t_dim
    m48 = a_end
    mv = m48 + CH * H
    wqt = mv + CH * H
    b_end = wqt + CH * D
    wvp = b_end
    wp = wvp + CH * D
    total = wp + CH * D
    return dict(WK=wk, KVT=kvt, M48=m48, MV=mv, A=a_end, WQT=wqt, B=b_end,
                WVP=wvp, WP=wp, TOTAL=total)


def build_nc(t_dim=T, n_dim=N):
    nc = bacc.Bacc("TRN2", target_bir_lowering=False, debug=False)
    off = _offsets(t_dim)

    nh = min(2 * NG, n_dim)  # columns per qt tile (half-t)
    gph = nh // NG  # groups per half
    nhalves = n_dim // nh
    groups = [
        (t, hf, gl) for t in range(t_dim) for hf in range(nhalves) for gl in range(gph)
    ]
    tiles_order = [(t, hf) for t in range(t_dim) for hf in range(nhalves)]
    G = len(groups)

    qT = nc.dram_tensor("qT", [t_dim, D, n_dim], BF16, kind="ExternalInput").ap()
    qT8 = nc.dram_tensor("qT8", [t_dim, D, n_dim], FP8, kind="ExternalInput").ap()
    cb16_d = nc.dram_tensor("cb16", [P, off["TOTAL"]], BF16, kind="ExternalInput").ap()
    c8_d = nc.dram_tensor("c8", [P, CH * D + CH * P], FP8, kind="ExternalInput").ap()
    outT = nc.dram_tensor("outT", [t_dim, D, n_dim], BF16, kind="ExternalOutput").ap()

    with tile.TileContext(nc) as tc, ExitStack() as ctx:
        consts = ctx.enter_context(tc.tile_pool(name="consts", bufs=1))
        qpool = ctx.enter_context(tc.tile_pool(name="qpool", bufs=4))
        q8pool = ctx.enter_context(tc.tile_pool(name="q8pool", bufs=4))
        work = ctx.enter_context(tc.tile_pool(name="work", bufs=2))
        qsqp = ctx.enter_context(tc.tile_pool(name="qsqp", bufs=3))
        nrmp = ctx.enter_context(tc.tile_pool(name="nrmp", bufs=2))
        attp = ctx.enter_context(tc.tile_pool(name="attp", bufs=4))
        osbp = ctx.enter_context(tc.tile_pool(name="osbp", bufs=6))
        pqp = ctx.enter_context(tc.tile_pool(name="pqp", bufs=3, space="PSUM"))
        ssp = ctx.enter_context(tc.tile_pool(name="ssp", bufs=1, space="PSUM"))
        parp = ctx.enter_context(tc.tile_pool(name="parp", bufs=1, space="PSUM"))
        pop = ctx.enter_context(tc.tile_pool(name="pop", bufs=1, space="PSUM"))
        dram = ctx.enter_context(tc.tile_pool(name="dram", bufs=1, space="DRAM"))

        # ---------- const + first-tile DMAs, priority order ----------
        cb16_t = consts.tile([P, off["TOTAL"]], BF16, tag="cb16")
        c8_t = consts.tile([P, CH * D + CH * P], FP8, tag="c8")
        wq8v = c8_t[:, 0 : CH * D].rearrange("p (c d) -> p c d", c=CH)
        # m48 padded to 128 stationary columns (cols 8.. are zero): DoubleRow
        # ldweights is only valid for full-width (0,0) tiles.
        m48v = c8_t[:, CH * D :].rearrange("p (c m) -> p c m", c=CH)

        qt_tiles = {}
        qt_parts = {}

        def emit_qdma(h, part=None):
            # part=None: whole tile; part=0: qt8 + first half of qt;
            # part=1: second half of qt (issued a round later to smooth
            # per-DMA-queue bursts that delay interleaved output DMAs)
            t, hf = h
            sl = slice(hf * nh, (hf + 1) * nh)
            if part in (None, 0):
                qt8 = q8pool.tile([P, CH, nh], FP8, tag="qt8")
                nc.sync.dma_start(
                    qt8, qT8[t].rearrange("(c p) n -> p c n", p=P)[:, :, sl]
                )
                qt = qpool.tile([P, CH, nh], BF16, tag="qt")
                qt_tiles[h] = (qt, qt8)
            qt = qt_tiles[h][0]
            hw = nh // 2
            src3 = qT[t].rearrange("(c p) n -> p c n", p=P)[:, :, sl]
            if part is None:
                nc.sync.dma_start(qt, src3)
            elif part == 0:
                nc.sync.dma_start(qt[:, :, 0:hw], src3[:, :, 0:hw])
            else:
                nc.sync.dma_start(qt[:, :, hw:], src3[:, :, hw:])
            qt_parts[h] = part

        emit_qdma(tiles_order[0])
        nc.sync.dma_start(c8_t, c8_d)
        nc.sync.dma_start(cb16_t[:, : off["A"]], cb16_d[:, : off["A"]])
        nc.sync.dma_start(cb16_t[:, off["A"] : off["B"]], cb16_d[:, off["A"] : off["B"]])

        def W(o, c, w):
            base = o + c * w
            return cb16_t[:, base : base + w]

        # ---------- PE warmup (HAM un-throttle while DMAs land) ----------
        wz = consts.tile([P, P], BF16, tag="wz")
        nc.vector.memset(wz, 0.0)
        # dummy sqrt: hoists the sqrt ACT-table load off the first round's
        # critical path (tables load on first use per function)
        dumq = consts.tile([1, 8], F32, tag="dumq")
        nc.vector.memset(dumq, 1.0)
        nc.scalar.sqrt(dumq, dumq)
        warm = pop.tile([H, P], F32, tag="po")

        def emit_warm(n):
            for _ in range(n):
                nc.tensor.matmul(
                    warm, wz[:, 0:H], wz, start=True, stop=True, skip_group_check=True
                )

        emit_warm(16)

        for k in (1, 2):
            if k < len(tiles_order):
                emit_qdma(tiles_order[k])
        nc.sync.dma_start(cb16_t[:, off["B"] :], cb16_d[:, off["B"] :])

        # ---------- per-group emission helpers ----------
        par_tiles = {}
        qsq_tiles = {}
        att_tiles = {}

        def emit_qproj_open(g):
            par = parp.tile([P, NG], F32, tag="par")
            qsq = qsqp.tile([P, CH, NG], FP8, tag="qsq")
            par_tiles[g] = par
            qsq_tiles[g] = qsq

        def emit_qproj_chunk(g, co):
            t, hf, gl = groups[g]
            qt, qt8 = qt_tiles[(t, hf)]
            qsl = slice(gl * NG, (gl + 1) * NG)
            qsq = qsq_tiles[g]
            pqc = pqp.tile([P, NG], F32, tag="pq")
            nc.tensor.matmul(
                pqc, wq8v[:, 0:2, co * P : (co + 1) * P], qt8[:, 0:2, qsl],
                start=True, stop=False, perf_mode=DR, skip_group_check=True,
            )
            nc.tensor.matmul(
                pqc, wq8v[:, 2, co * P : (co + 1) * P], qt8[:, 2, qsl],
                start=False, stop=True, skip_group_check=True,
            )
            nc.scalar.square(qsq[:, co, :], pqc)

        def emit_qproj(g):
            emit_qproj_open(g)
            for co in range(CH):
                emit_qproj_chunk(g, co)

        RW = 64 + H  # replicated stationary width: strips at 0, 32, 64

        def emit_attn(g):
            t, hf, gl = groups[g]
            qt, _ = qt_tiles[(t, hf)]
            qsl = slice(gl * NG, (gl + 1) * NG)
            par = par_tiles[g]
            for c in range(CH):
                nc.tensor.matmul(
                    par[0:RW, :], wqk[:, c, t, :], qt[:, c, qsl],
                    start=(c == 0), stop=(c == CH - 1),
                    tile_position=(0, 0),
                )

        rs_tiles = {}

        def emit_stageA1(g):
            qsq = qsq_tiles.pop(g)
            sspq = ssp.tile([P, NG], F32, tag="ss")
            nc.tensor.matmul(
                sspq, m48v[:, 0:2, :], qsq[:, 0:2, :],
                start=True, stop=False, perf_mode=DR, skip_group_check=True,
            )
            nc.tensor.matmul(
                sspq, m48v[:, 2, :], qsq[:, 2, :],
                start=False, stop=True, skip_group_check=True,
            )
            # rcp first: frees the ss psum bank after a single DVE op
            rs = nrmp.tile([RW, NG], F32, tag="rs")
            nc.vector.reciprocal_approx_fast(rs, sspq[0:RW, :])
            rs_tiles[g] = rs

        def emit_stageA2(g):
            # att = attn_raw * sqrt(1/ss); 1/|k_h| is folded into Wp_eff.
            # Everything is strip-replicated on partitions [0:72], so att
            # comes out pre-replicated for the row-tiled out MMs.
            par = par_tiles.pop(g)
            rs = rs_tiles.pop(g)
            nc.scalar.sqrt(rs, rs)
            att = attp.tile([P, NG], BF16, tag="att")
            nc.vector.tensor_tensor(att[0:RW, :], par[0:RW, :], rs, op=mybir.AluOpType.mult)
            att_tiles[g] = att

        def emit_stageA(g):
            emit_stageA1(g)
            emit_stageA2(g)

        osb_cur = [None]

        def emit_stageB(g):
            t, hf, gl = groups[g]
            att = att_tiles.pop(g)
            po = pop.tile([P, CH, NG], F32, tag="po")
            for r in range(CH):
                nc.tensor.matmul(
                    po[:, r, :],
                    wpe_srep[32 * r : 32 * r + H, t, r * P : (r + 1) * P],
                    att[32 * r : 32 * r + H, :],
                    start=True, stop=True, skip_group_check=True,
                )
            # pair-batched osb: both groups of a (t,hf) tile share one tile,
            # flushed with a single 2KB-row DMA (DMA descriptor efficiency)
            if gl == 0 or osb_cur[0] is None:
                osbt = osbp.tile([P, CH, gph * NG], BF16, tag="osb")
                osb_cur[0] = osbt
            osb = osb_cur[0]
            gsl = slice(gl * NG, (gl + 1) * NG)
            nc.scalar.copy(osb[:, :, gl * NG + ES : (gl + 1) * NG], po[:, :, ES:])
            nc.vector.tensor_copy(osb[:, :, gl * NG : gl * NG + ES], po[:, :, 0:ES])
            if gl == gph - 1:
                sl = slice(hf * nh, (hf + 1) * nh)
                dst = outT[t].rearrange("(c p) n -> p c n", p=P)[:, :, sl]
                nc.sync.dma_start(dst, osb)
                osb_cur[0] = None

        # ---------- preamble ----------
        PRE = min(2, G)
        emit_qproj(0)

        # k projection computed directly in transposed (D-part, t) form:
        # stationary = weight chunk, moving = kvT (t rows) - no transposes.
        kT = consts.tile([P, CH, t_dim], BF16, tag="kT")
        for co in range(CH):
            pk = pqp.tile([P, t_dim], F32, tag="pq")
            for c in range(CH):
                nc.tensor.matmul(
                    pk, W(off["WK"], c, D)[:, co * P : (co + 1) * P],
                    W(off["KVT"], c, t_dim),
                    start=(c == 0), stop=(c == CH - 1),
                )
            nc.vector.tensor_copy(kT[:, co, :], pk)

        # per-head k norms: rnkT[h, t] = 1/|k_h|(t)
        ksqT = work.tile([P, CH, t_dim], BF16, tag="ksqT")
        nc.scalar.square(ksqT, kT)
        psk2 = pqp.tile([H, t_dim], F32, tag="pq")
        for c in range(CH):
            nc.tensor.matmul(
                psk2, W(off["M48"], c, H), ksqT[:, c, :],
                start=(c == 0), stop=(c == CH - 1),
            )
        rnkT = consts.tile([H, t_dim], F32, tag="rnkT")
        nc.scalar.sqrt(rnkT, psk2)
        nc.vector.reciprocal(rnkT, rnkT)
        emit_warm(8)

        if PRE > 1:
            emit_qproj(1)

        # Kmat[d, t, h] = kT[d, t] * m48[d, h]
        m48b = cb16_t[:, off["M48"] : off["M48"] + CH * H].rearrange(
            "p (c h) -> p c h", c=CH
        )
        mvb = cb16_t[:, off["MV"] : off["MV"] + CH * H].rearrange(
            "p (c h) -> p c h", c=CH
        )
        kmat = consts.tile([P, CH, t_dim, H], BF16, tag="kmat")
        for h in range(H):
            nc.vector.tensor_tensor(
                kmat[:, :, :, h],
                kT,
                m48b[:, :, h : h + 1].to_broadcast((P, CH, t_dim)),
                op=mybir.AluOpType.mult,
            )

        # Wqk[d_in, t, h] = sum_dmid Wq[d_in, dmid] Kmat[dmid, t, h]
        # Stationary columns replicated to strips {0-7, 32-39, 64-71} so the
        # attn matmuls write all three partition strips at once; gaps zero.
        wqk = consts.tile([P, CH, t_dim, RW], BF16, tag="wqk")
        nc.gpsimd.memset(wqk, 0.0)
        for ci in range(CH):
            pw = pqp.tile([P, t_dim * H], F32, tag="pq")
            for cm in range(CH):
                nc.tensor.matmul(
                    pw,
                    W(off["WQT"], cm, D)[:, ci * P : (ci + 1) * P],
                    kmat[:, cm, :, :],
                    start=(cm == 0), stop=(cm == CH - 1),
                )
            pwv = pw.rearrange("p (t h) -> p t h", h=H)
            for s in range(3):
                nc.vector.tensor_copy(wqk[:, ci, :, 32 * s : 32 * s + H], pwv)

        emit_warm(8)
        for g in range(PRE):
            emit_attn(g)

        # v projection (transposed) + Vsel
        vT = consts.tile([P, CH, t_dim], BF16, tag="vT")
        for co in range(CH):
            pv = pqp.tile([P, t_dim], F32, tag="pq")
            for c in range(CH):
                nc.tensor.matmul(
                    pv, W(off["WVP"], c, D)[:, co * P : (co + 1) * P],
                    W(off["KVT"], c, t_dim),
                    start=(c == 0), stop=(c == CH - 1),
                )
            nc.vector.tensor_copy(vT[:, co, :], pv)
        vsel = consts.tile([P, CH, t_dim, H], BF16, tag="vsel")
        for h in range(H):
            nc.vector.tensor_tensor(
                vsel[:, :, :, h],
                vT,
                mvb[:, :, h : h + 1].to_broadcast((P, CH, t_dim)),
                op=mybir.AluOpType.mult,
            )

        # Wp_eff[(t,h), d_out] = sum_d Vsel[d, t, h] * Wp[d, d_out]
        pe_all = pop.tile([t_dim * H, D], F32, tag="po")
        for c in range(CH):
            nc.tensor.matmul(
                pe_all, vsel[:, c].rearrange("p t h -> p (t h)"), W(off["WP"], c, D),
                start=(c == 0), stop=(c == CH - 1),
            )
        wpe_stage = work.tile([t_dim * H, D], BF16, tag="wpestage")
        nc.scalar.copy(wpe_stage, pe_all)
        wpe_dram = dram.tile([t_dim * H, D], BF16)
        nc.gpsimd.dma_start(wpe_dram, wpe_stage)
        wpe = consts.tile([H, t_dim, D], BF16, tag="wpe")
        nc.gpsimd.dma_start(wpe, wpe_dram.rearrange("(t h) d -> h t d", h=H))
        # fold 1/|k_h| into the output projection; replicate to row strips
        wpe_srep = consts.tile([P, t_dim, D], BF16, tag="wpes")
        nc.vector.tensor_tensor(
            wpe_srep[0:H], wpe, rnkT[:, :, None].to_broadcast((H, t_dim, D)),
            op=mybir.AluOpType.mult,
        )
        nc.gpsimd.dma_start(wpe_srep[32 : 32 + H], wpe_srep[0:H])
        nc.gpsimd.dma_start(wpe_srep[64 : 64 + H], wpe_srep[0:H])

        if PRE > 1:
            emit_stageA(0)

        # ---------- software-pipelined main loop ----------
        LAGB = 2
        NTAIL = min(4, G)  # last groups drain per-chunk through the pq ring
        if (G - NTAIL) % gph:
            NTAIL += 1  # keep the body/tail boundary pair-aligned
        for g in range(PRE, G):
            t, hf, gl = groups[g]
            if gl == 0:
                k = g // gph + 2  # prefetch two (t,half) tiles ahead
                if k < len(tiles_order) and tiles_order[k] not in qt_tiles:
                    emit_qdma(tiles_order[k])
            emit_stageA1(g - 1)
            if 0 <= g - LAGB < G - NTAIL:
                emit_stageB(g - LAGB)
            emit_qproj_open(g)
            emit_qproj_chunk(g, 0)
            emit_qproj_chunk(g, 1)
            emit_stageA2(g - 1)
            emit_qproj_chunk(g, 2)
            emit_attn(g)
        def emit_stageB_tail(g):
            t, hf, gl = groups[g]
            sl = slice(hf * nh + gl * NG, hf * nh + (gl + 1) * NG)
            att = att_tiles.pop(g)
            osb = osbp.tile([P, CH, NG], BF16, tag="osb")
            dst = outT[t].rearrange("(c p) n -> p c n", p=P)[:, :, sl]
            for r in range(CH):
                poc = pqp.tile([P, NG], F32, tag="pq")
                nc.tensor.matmul(
                    poc,
                    wpe_srep[32 * r : 32 * r + H, t, r * P : (r + 1) * P],
                    att[32 * r : 32 * r + H, :],
                    start=True, stop=True, skip_group_check=True,
                )
                if r == 1:
                    nc.vector.tensor_copy(osb[:, r, :], poc)
                else:
                    nc.scalar.copy(osb[:, r, :], poc)
                eng = (nc.gpsimd, nc.sync, nc.gpsimd)[r]
                eng.dma_start(dst[:, r, :], osb[:, r, :])

        emit_stageA(G - 1)
        for g in range(G - NTAIL, G):
            emit_stageB_tail(g)

    nc.compile()
    return nc


_CACHE = {}


def _get_nc(t_dim=T, n_dim=N):
    key = (t_dim, n_dim)
    if key not in _CACHE:
        _CACHE[key] = build_nc(t_dim, n_dim)
    return _CACHE[key]


def _host_prep(query, kv, Wq, Wk, Wv, Wp, bp):
    bf = ml_dtypes.bfloat16
    f8 = ml_dtypes.float8_e4m3fn
    query = np.asarray(query, dtype=np.float32)
    kv = np.asarray(kv, dtype=np.float32)
    Wq = np.asarray(Wq, dtype=np.float32)
    Wk = np.asarray(Wk, dtype=np.float32)
    Wv = np.asarray(Wv, dtype=np.float32)
    Wp = np.asarray(Wp, dtype=np.float32)
    bp = np.asarray(bp, dtype=np.float32)

    b_dim, t_dim, n_dim, d = query.shape
    dh = d // H
    off = _offsets(t_dim)

    def img(mat, width):  # [D, width] -> [P, CH*width] chunk-major image
        return np.ascontiguousarray(
            mat.reshape(CH, P, width).transpose(1, 0, 2).reshape(P, CH * width)
        )

    perm = (np.arange(d) % H) * dh + np.arange(d) // H
    Wvp = Wv[:, perm]
    dd = np.arange(d)
    hh = np.arange(H)
    m48 = (dd[:, None] // dh == hh[None, :]).astype(np.float32)
    mv = (dd[:, None] % H == hh[None, :]).astype(np.float32)

    def reg(base, w):
        return slice(base, base + w)

    base = np.empty((P, off["TOTAL"]), dtype=bf)
    base[:, reg(off["WK"], CH * d)] = img(Wk, d).astype(bf)
    base[:, reg(off["M48"], CH * H)] = img(m48, H).astype(bf)
    base[:, reg(off["MV"], CH * H)] = img(mv, H).astype(bf)
    base[:, reg(off["WQT"], CH * d)] = img(np.ascontiguousarray(Wq.T), d).astype(bf)
    base[:, reg(off["WVP"], CH * d)] = img(Wvp, d).astype(bf)
    base[:, reg(off["WP"], CH * d)] = img(Wp, d).astype(bf)

    # ss masks strip-replicated at cols {0-7, 32-39, 64-71}; every gap
    # column carries head 0's mask so 1/ss never sees a zero.
    m48pad = np.tile(m48[:, 0:1], (1, P)).astype(np.float32)
    for s in range(3):
        m48pad[:, 32 * s : 32 * s + H] = m48
    c8 = np.empty((P, CH * d + CH * P), dtype=f8)
    c8[:, : CH * d] = img(Wq, d).astype(f8)
    c8[:, CH * d :] = img(m48pad, P).astype(f8)

    in_maps = []
    for b in range(b_dim):
        cb16 = base.copy()
        cb16[:, reg(off["KVT"], CH * t_dim)] = img(kv[b, :, 0, :].T, t_dim).astype(bf)
        qTb = np.ascontiguousarray(query[b].transpose(0, 2, 1))
        in_maps.append(
            {
                "qT": qTb.astype(bf),
                "qT8": qTb.astype(f8),
                "cb16": cb16,
                "c8": c8,
            }
        )
    return in_maps, (b_dim, t_dim, n_dim, d), bp


def _gather(results, shape, bp):
    b_dim, t_dim, n_dim, d = shape
    out = np.empty((b_dim, t_dim, n_dim, d), dtype=np.float32)
    for b in range(b_dim):
        out[b] = results[b]["outT"].astype(np.float32).transpose(0, 2, 1)
    if np.any(bp):
        out += bp
    return out


def kernel(query, kv, Wq, Wk, Wv, Wp, bp):
    from concourse.bass_utils import run_bass_kernel_spmd

    in_maps, shape, bp = _host_prep(query, kv, Wq, Wk, Wv, Wp, bp)
    nc = _get_nc(shape[1], shape[2])
    res = run_bass_kernel_spmd(nc, in_maps, core_ids=list(range(len(in_maps))))
    return _gather(res.results, shape, bp)


def _install_ntff_hook():
    """The agent image's antenv lacks axon_hooks; synthesize it so
    run_bass_kernel_spmd(trace=True) can capture NTFF profiles."""
    import types

    if "antenv.axon_hooks" in sys.modules:
        return
    sys.path.insert(0, "/root/.axon_site")
    from trn_agent_boot.trn_boot import _ntff_profile_via_ctypes

    hook = _ntff_profile_via_ctypes("/opt/axon/libaxon_pjrt.so")
    mod = types.ModuleType("antenv.axon_hooks")
    mod.get_axon_ntff_profile_hook = lambda: hook
    mod.set_axon_ntff_profile_hook = lambda h: None
    sys.modules["antenv.axon_hooks"] = mod


def kernel_traced(query, kv, Wq, Wk, Wv, Wp, bp):
    """Like kernel() but captures an NTFF profile; returns (out, results)."""
    from concourse.bass_utils import run_bass_kernel_spmd

    _install_ntff_hook()
    in_maps, shape, bp = _host_prep(query, kv, Wq, Wk, Wv, Wp, bp)
    nc = _get_nc(shape[1], shape[2])
    res = run_bass_kernel_spmd(
        nc, in_maps, core_ids=list(range(len(in_maps))), trace=True
    )
    return _gather(res.results, shape, bp), res
